# revision 15
# baseline (speedup 1.0000x reference)
"""Trainium2 Bass kernel for GreedyGroupedRouter (MoE routing).

Reference computation per token (row of 256 logits):
  rw   = softmax(logits)                          [S, 256]
  per group g of 32 experts: pick argmax           -> topk_ids [S, 8]
  tw   = gathered rw, renormalized to sum 1        [S, 8]
  cnt  = bincount(topk_ids, 256)                   [256]
Outputs: (logits, rw, tw, ids, cnt).  logits is a pass-through.

Sharding: token dim across 8 cores (16384 tokens/core), SPMD identical
program; tokens_per_expert partial counts summed on host.

Per-core program: 32 "big tiles" of 512 tokens laid out as SBUF
[128 partitions, 4 sub-rows, 256 experts] (token = p*128 + b*4 + q, i.e.
partition-major so every DMA run is >=4KB contiguous):
  DVE : group-max (8 groups of 32), max_index -> global expert ids,
        reciprocals
  ACT : exp (softmax numerator; no max-subtract needed, |logits|<~6),
        accum_out gives row sums for free, scaled copies
  Pool: is_equal(logits, groupmax broadcast) -> one-hot selection mask
  PE  : ones[128,1].T @ mask accumulated in PSUM -> per-expert counts
  small outputs (tw, ids) accumulate in SBUF, one big DMA at the end.
"""

import os
import sys

import numpy as np

sys.path.insert(0, "/opt/trn_rl_repo")

import concourse.bacc as bacc
import concourse.bass as bass
import concourse.tile as tile
from concourse import mybir
from concourse._compat import get_trn_type
from concourse.bass_utils import run_bass_kernel_spmd

SEQ = 131072
N_EXPERTS = 256
N_GROUPS = 8
GROUP_SIZE = 32
TOP_K = 8
N_CORES = 8
TOK_PER_CORE = SEQ // N_CORES  # 16384
Q = 4  # sub-rows per partition per big tile
P = 128
BIG_TOK = P * Q  # 512 tokens per big tile
N_BIG = TOK_PER_CORE // BIG_TOK  # 32
N_LOC = TOK_PER_CORE // P  # 128 tokens per partition

F32 = mybir.dt.float32
U32 = mybir.dt.uint32
U16 = mybir.dt.uint16
BF16 = mybir.dt.bfloat16
I32 = mybir.dt.int32
AX = mybir.AxisListType
ALU = mybir.AluOpType
ACTF = mybir.ActivationFunctionType


def _bcast_last(ap, n):
    """Append a broadcast (step 0) innermost dim of size n to an AP."""
    return bass.AP(tensor=ap.tensor, offset=ap.offset, ap=list(ap.ap) + [[0, n]])


def _build_program():
    nc = bacc.Bacc(
        get_trn_type() or "TRN2",
        target_bir_lowering=False,
        debug=False,
        num_devices=N_CORES,
    )

    logits_d = nc.dram_tensor("logits", [TOK_PER_CORE, N_EXPERTS], F32, kind="ExternalInput")
    rw_d = nc.dram_tensor("rw", [TOK_PER_CORE, N_EXPERTS], F32, kind="ExternalOutput")
    tw_d = nc.dram_tensor("tw", [TOK_PER_CORE, TOP_K], F32, kind="ExternalOutput")
    ids_d = nc.dram_tensor("ids", [TOK_PER_CORE, TOP_K], U16, kind="ExternalOutput")
    # 4 sub-row blocks of partial counts; host sums them
    cnt_d = nc.dram_tensor("cnt", [1, Q * N_EXPERTS], F32, kind="ExternalOutput")

    # token t = p*N_LOC + n  (partition-major)  ->  dram row view [p, n, e]
    lg_v = logits_d.ap().rearrange("(p n) e -> p n e", p=P)
    rw_v = rw_d.ap().rearrange("(p n) e -> p n e", p=P)
    tw_v = tw_d.ap().rearrange("(p n) g -> p n g", p=P)
    ids_v = ids_d.ap().rearrange("(p n) g -> p n g", p=P)

    with tile.TileContext(nc) as tc:
        with (
            tc.tile_pool(name="singles", bufs=1) as singles,
            tc.tile_pool(name="acc", bufs=1) as acc,
            tc.tile_pool(name="big", bufs=3) as bigp,
            tc.tile_pool(name="small", bufs=3) as smallp,
            tc.tile_pool(name="psum", bufs=1, space="PSUM") as psump,
        ):
            ones = singles.tile([P, 1], BF16)
            nc.vector.memset(ones, 1.0)
            # global expert index per row position, same on every partition
            iota = singles.tile([P, N_EXPERTS], U16)
            nc.gpsimd.iota(iota, pattern=[[1, N_EXPERTS]], base=0, channel_multiplier=0)

            tw_acc = acc.tile([P, N_LOC, TOP_K], F32)
            ids_acc = acc.tile([P, N_LOC, TOP_K], U16)
            cnt_psA = psump.tile([1, 2 * N_EXPERTS], F32)
            cnt_psB = psump.tile([1, 2 * N_EXPERTS], F32)

            for b in range(N_BIG):
                L = bigp.tile([P, Q, N_EXPERTS], F32, tag="L")
                nc.sync.dma_start(out=L, in_=lg_v[:, b * Q : (b + 1) * Q, :])

                # group maxes for all Q sub-rows: [P, Q*8]
                GM = smallp.tile([P, Q * N_GROUPS], F32, tag="GM")
                nc.vector.reduce_max(
                    GM,
                    L[:, :, :].rearrange("p q (g s) -> p q g s", s=GROUP_SIZE),
                    axis=AX.X,
                )

                E = bigp.tile([P, Q, N_EXPERTS], F32, tag="E")
                rowsum = smallp.tile([P, Q], F32, tag="rowsum")
                for q in range(Q):
                    nc.scalar.activation(
                        E[:, q, :], L[:, q, :], ACTF.Exp,
                        accum_out=rowsum[:, q : q + 1],
                    )

                recip = smallp.tile([P, Q], F32, tag="recip")
                nc.vector.reciprocal(recip, rowsum)

                RW = bigp.tile([P, Q, N_EXPERTS], F32, tag="RW")
                for q in range(Q):
                    nc.gpsimd.tensor_scalar_mul(
                        RW[:, q, :], E[:, q, :], recip[:, q : q + 1]
                    )
                nc.sync.dma_start(out=rw_v[:, b * Q : (b + 1) * Q, :], in_=RW)

                # global expert ids: first index of each group max in the row
                for q in range(Q):
                    nc.vector.max_index(
                        ids_acc[:, b * Q + q, :],
                        GM[:, q * N_GROUPS : (q + 1) * N_GROUPS],
                        L[:, q, :],
                    )

                # normalized gathered weights: exp(gm) / sum_g exp(gm)
                EG = smallp.tile([P, Q * N_GROUPS], F32, tag="EG")
                nc.scalar.activation(EG, GM, ACTF.Exp)
                s8 = smallp.tile([P, Q], F32, tag="s8")
                nc.vector.reduce_sum(
                    s8, EG[:, :].rearrange("p (q g) -> p q g", g=N_GROUPS), axis=AX.X
                )
                r8 = smallp.tile([P, Q], F32, tag="r8")
                nc.vector.reciprocal(r8, s8)
                nc.vector.tensor_tensor(
                    tw_acc[:, b * Q : (b + 1) * Q, :],
                    EG[:, :].rearrange("p (q g) -> p q g", g=N_GROUPS),
                    _bcast_last(r8[:, :], N_GROUPS),
                    ALU.mult,
                )

                # exact one-hot of the chosen ids (== bincount of ids once
                # summed): cmp[p,q,g,s] = (32*g + s == ids[p,q,g])
                cmp = bigp.tile([P, Q, N_EXPERTS], BF16, tag="cmp")
                iota_ap = iota[:, :]
                iota_b = bass.AP(
                    tensor=iota_ap.tensor,
                    offset=iota_ap.offset,
                    ap=[iota_ap.ap[0], [0, Q], [GROUP_SIZE, N_GROUPS], [1, GROUP_SIZE]],
                )
                ids_b = _bcast_last(
                    ids_acc[:, b * Q : (b + 1) * Q, :], GROUP_SIZE
                )
                nc.vector.tensor_tensor(
                    cmp[:, :, :].rearrange("p q (g s) -> p q g s", s=GROUP_SIZE),
                    iota_b,
                    ids_b,
                    ALU.is_equal,
                )
                nc.tensor.matmul(
                    cnt_psA, ones, cmp[:, 0:2, :].rearrange("p q e -> p (q e)"),
                    start=(b == 0), stop=(b == N_BIG - 1),
                )
                nc.tensor.matmul(
                    cnt_psB, ones, cmp[:, 2:4, :].rearrange("p q e -> p (q e)"),
                    start=(b == 0), stop=(b == N_BIG - 1),
                )

            nc.sync.dma_start(out=tw_v, in_=tw_acc)
            nc.sync.dma_start(out=ids_v, in_=ids_acc)

            cnt_sb = singles.tile([1, Q * N_EXPERTS], F32)
            nc.vector.tensor_copy(cnt_sb[:, 0 : 2 * N_EXPERTS], cnt_psA)
            nc.vector.tensor_copy(cnt_sb[:, 2 * N_EXPERTS : 4 * N_EXPERTS], cnt_psB)
            nc.sync.dma_start(out=cnt_d.ap(), in_=cnt_sb)

    nc.compile()
    return nc


def _ensure_ntff_hook():
    """The agent image's antenv lacks axon_hooks; fabricate it and register
    the ctypes NTFF hook so trace=True produces a profile under axon."""
    import types

    import antenv

    if "antenv.axon_hooks" in sys.modules:
        return
    sys.path.insert(0, "/root/.axon_site")
    from trn_agent_boot.trn_boot import _ntff_profile_via_ctypes

    mod = types.ModuleType("antenv.axon_hooks")
    holder = [None]
    mod.set_axon_ntff_profile_hook = lambda h: holder.__setitem__(0, h)
    mod.get_axon_ntff_profile_hook = lambda: holder[0]
    sys.modules["antenv.axon_hooks"] = mod
    antenv.axon_hooks = mod
    mod.set_axon_ntff_profile_hook(
        _ntff_profile_via_ctypes("/opt/axon/libaxon_pjrt.so")
    )


_NC_CACHE = None


def _get_nc():
    global _NC_CACHE
    if _NC_CACHE is None:
        _NC_CACHE = _build_program()
    return _NC_CACHE


def _run(logits, trace=False, tmpdir=None):
    logits = np.ascontiguousarray(np.asarray(logits, dtype=np.float32))
    assert logits.shape == (SEQ, N_EXPERTS)
    nc = _get_nc()
    if trace:
        _ensure_ntff_hook()
    # core c gets tokens [c*16384, (c+1)*16384); inside, token t of the
    # shard lives at dram row t, program reads it partition-major.
    in_maps = [
        {"logits": logits[c * TOK_PER_CORE : (c + 1) * TOK_PER_CORE]}
        for c in range(N_CORES)
    ]
    res = run_bass_kernel_spmd(
        nc, in_maps, list(range(N_CORES)), trace=trace, tmpdir=tmpdir
    )
    outs = res.results
    rw = np.concatenate([r["rw"] for r in outs], axis=0)
    tw = np.concatenate([r["tw"] for r in outs], axis=0)
    ids = np.concatenate([r["ids"] for r in outs], axis=0).astype(np.int32)
    cnt = np.sum(
        [r["cnt"].reshape(Q, N_EXPERTS).sum(axis=0) for r in outs],
        axis=0,
        dtype=np.float64,
    ).reshape(N_EXPERTS)

    # max_index searches the whole 256-wide row, so an exact f32 duplicate of
    # a group max located in a *different* group can steal the match. Such an
    # id is out of its group -> detectable; fix id and count delta exactly.
    grp = np.arange(N_GROUPS, dtype=np.int32)[None, :]
    bad_r, bad_g = np.nonzero((ids >> 5) != grp)
    for r, g in zip(bad_r.tolist(), bad_g.tolist()):
        seg = logits[r, g * GROUP_SIZE : (g + 1) * GROUP_SIZE]
        new = g * GROUP_SIZE + int(np.argmax(seg))
        ids[r, g] = new
        # the out-of-group id never matched iota in group g's slice, so the
        # device counted nothing for this slot: just add the fixed one.
        cnt[new] += 1.0

    cnt = cnt.astype(np.float32)
    return (logits, rw, tw, ids, cnt), res


def kernel(**inputs):
    (out, _res) = _run(inputs["logits"], trace=False)
    return out


# revision 18
# speedup vs baseline: 3.2398x; 3.2398x over previous
"""Trainium2 Bass kernel for GreedyGroupedRouter (MoE routing).

Reference computation per token (row of 256 logits):
  rw   = softmax(logits)                          [S, 256]
  per group g of 32 experts: pick argmax           -> topk_ids [S, 8]
  tw   = gathered rw, renormalized to sum 1        [S, 8]
  cnt  = bincount(topk_ids, 256)                   [256]
Outputs: (logits, rw, tw, ids, cnt).  logits is a pass-through.

Sharding: token dim across 8 cores (16384 tokens/core), SPMD identical
program; tokens_per_expert partial counts summed on host.

Per-core program: 32 "big tiles" of 512 tokens laid out as SBUF
[128 partitions, 4 sub-rows, 256 experts] (token = p*128 + b*4 + q, i.e.
partition-major so every DMA run is >=4KB contiguous):
  DVE : group-max (8 groups of 32), max_index -> global expert ids,
        reciprocals
  ACT : exp (softmax numerator; no max-subtract needed, |logits|<~6),
        accum_out gives row sums for free, scaled copies
  Pool: is_equal(logits, groupmax broadcast) -> one-hot selection mask
  PE  : ones[128,1].T @ mask accumulated in PSUM -> per-expert counts
  small outputs (tw, ids) accumulate in SBUF, one big DMA at the end.
"""

import os
import sys

import numpy as np

sys.path.insert(0, "/opt/trn_rl_repo")

import concourse.bacc as bacc
import concourse.bass as bass
import concourse.tile as tile
from concourse import mybir
from concourse._compat import get_trn_type
from concourse.bass_utils import run_bass_kernel_spmd

SEQ = 131072
N_EXPERTS = 256
N_GROUPS = 8
GROUP_SIZE = 32
TOP_K = 8
N_CORES = 8
TOK_PER_CORE = SEQ // N_CORES  # 16384
Q = 4  # sub-rows per partition per big tile
P = 128
BIG_TOK = P * Q  # 512 tokens per big tile
N_BIG = TOK_PER_CORE // BIG_TOK  # 32
N_LOC = TOK_PER_CORE // P  # 128 tokens per partition

F32 = mybir.dt.float32
U32 = mybir.dt.uint32
U16 = mybir.dt.uint16
BF16 = mybir.dt.bfloat16
I32 = mybir.dt.int32
AX = mybir.AxisListType
ALU = mybir.AluOpType
ACTF = mybir.ActivationFunctionType


def _bcast_last(ap, n):
    """Append a broadcast (step 0) innermost dim of size n to an AP."""
    return bass.AP(tensor=ap.tensor, offset=ap.offset, ap=list(ap.ap) + [[0, n]])


def _build_program():
    nc = bacc.Bacc(
        get_trn_type() or "TRN2",
        target_bir_lowering=False,
        debug=False,
        num_devices=N_CORES,
    )

    logits_d = nc.dram_tensor("logits", [TOK_PER_CORE, N_EXPERTS], F32, kind="ExternalInput")
    rw_d = nc.dram_tensor("rw", [TOK_PER_CORE, N_EXPERTS], F32, kind="ExternalOutput")
    tw_d = nc.dram_tensor("tw", [TOK_PER_CORE, TOP_K], F32, kind="ExternalOutput")
    ids_d = nc.dram_tensor("ids", [TOK_PER_CORE, TOP_K], U16, kind="ExternalOutput")
    # 4 sub-row blocks of partial counts; host sums them
    cnt_d = nc.dram_tensor("cnt", [1, Q * N_EXPERTS], F32, kind="ExternalOutput")

    # token t = p*N_LOC + n  (partition-major)  ->  dram row view [p, n, e]
    lg_v = logits_d.ap().rearrange("(p n) e -> p n e", p=P)
    rw_v = rw_d.ap().rearrange("(p n) e -> p n e", p=P)
    tw_v = tw_d.ap().rearrange("(p n) g -> p n g", p=P)
    ids_v = ids_d.ap().rearrange("(p n) g -> p n g", p=P)

    with tile.TileContext(nc) as tc:
        with (
            tc.tile_pool(name="singles", bufs=1) as singles,
            tc.tile_pool(name="acc", bufs=1) as acc,
            tc.tile_pool(name="big", bufs=3) as bigp,
            tc.tile_pool(name="small", bufs=3) as smallp,
            tc.tile_pool(name="psum", bufs=1, space="PSUM") as psump,
        ):
            ones = singles.tile([P, 1], BF16)
            nc.vector.memset(ones, 1.0)
            # global expert index per row position, same on every partition
            iota_u = singles.tile([P, N_EXPERTS], U16)
            nc.gpsimd.iota(iota_u, pattern=[[1, N_EXPERTS]], base=0, channel_multiplier=0)
            iota = singles.tile([P, N_EXPERTS], BF16)
            nc.vector.tensor_copy(iota, iota_u)

            tw_acc = acc.tile([P, N_LOC, TOP_K], F32)
            ids_acc = acc.tile([P, N_LOC, TOP_K], U16)
            cnt_psA = psump.tile([1, 2 * N_EXPERTS], F32)
            cnt_psB = psump.tile([1, 2 * N_EXPERTS], F32)

            for b in range(N_BIG):
                L = bigp.tile([P, Q, N_EXPERTS], F32, tag="L")
                nc.sync.dma_start(out=L, in_=lg_v[:, b * Q : (b + 1) * Q, :])

                # group maxes for all Q sub-rows: [P, Q*8]
                GM = smallp.tile([P, Q * N_GROUPS], F32, tag="GM")
                nc.vector.reduce_max(
                    GM,
                    L[:, :, :].rearrange("p q (g s) -> p q g s", s=GROUP_SIZE),
                    axis=AX.X,
                )

                E = bigp.tile([P, Q, N_EXPERTS], F32, tag="E")
                rowsum = smallp.tile([P, Q], F32, tag="rowsum")
                for q in range(Q):
                    nc.scalar.activation(
                        E[:, q, :], L[:, q, :], ACTF.Exp,
                        accum_out=rowsum[:, q : q + 1],
                    )

                recip = smallp.tile([P, Q], F32, tag="recip")
                nc.vector.reciprocal(recip, rowsum)

                RW = bigp.tile([P, Q, N_EXPERTS], F32, tag="RW")
                for q in range(Q):
                    # balance the normalization multiply across ACT and DVE
                    if q < 3:
                        nc.scalar.activation(
                            RW[:, q, :], E[:, q, :], ACTF.Copy,
                            scale=recip[:, q : q + 1],
                        )
                    else:
                        nc.vector.tensor_scalar_mul(
                            RW[:, q, :], E[:, q, :], recip[:, q : q + 1]
                        )
                nc.sync.dma_start(out=rw_v[:, b * Q : (b + 1) * Q, :], in_=RW)

                # global expert ids: first index of each group max in the row
                for q in range(Q):
                    nc.vector.max_index(
                        ids_acc[:, b * Q + q, :],
                        GM[:, q * N_GROUPS : (q + 1) * N_GROUPS],
                        L[:, q, :],
                    )

                # normalized gathered weights: exp(gm) / sum_g exp(gm)
                EG = smallp.tile([P, Q * N_GROUPS], F32, tag="EG")
                nc.scalar.activation(EG, GM, ACTF.Exp)
                s8 = smallp.tile([P, Q], F32, tag="s8")
                nc.vector.reduce_sum(
                    s8, EG[:, :].rearrange("p (q g) -> p q g", g=N_GROUPS), axis=AX.X
                )
                r8 = smallp.tile([P, Q], F32, tag="r8")
                nc.vector.reciprocal(r8, s8)
                nc.vector.tensor_tensor(
                    tw_acc[:, b * Q : (b + 1) * Q, :],
                    EG[:, :].rearrange("p (q g) -> p q g", g=N_GROUPS),
                    _bcast_last(r8[:, :], N_GROUPS),
                    ALU.mult,
                )

                # exact one-hot of the chosen ids (== bincount of ids once
                # summed): cmp[p,q,g,s] = (32*g + s == ids[p,q,g])
                cmp = bigp.tile([P, Q, N_EXPERTS], BF16, tag="cmp")
                iota_ap = iota[:, :]
                iota_b = bass.AP(
                    tensor=iota_ap.tensor,
                    offset=iota_ap.offset,
                    ap=[iota_ap.ap[0], [0, Q], [GROUP_SIZE, N_GROUPS], [1, GROUP_SIZE]],
                )
                ids_bf = smallp.tile([P, Q * TOP_K], BF16, tag="ids_bf")
                nc.vector.tensor_copy(
                    ids_bf,
                    ids_acc[:, b * Q : (b + 1) * Q, :].rearrange("p q g -> p (q g)"),
                )
                ids_b = _bcast_last(
                    ids_bf[:, :].rearrange("p (q g) -> p q g", g=N_GROUPS), GROUP_SIZE
                )
                nc.vector.tensor_tensor(
                    cmp[:, :, :].rearrange("p q (g s) -> p q g s", s=GROUP_SIZE),
                    iota_b,
                    ids_b,
                    ALU.is_equal,
                )
                nc.tensor.matmul(
                    cnt_psA, ones, cmp[:, 0:2, :].rearrange("p q e -> p (q e)"),
                    start=(b == 0), stop=(b == N_BIG - 1),
                )
                nc.tensor.matmul(
                    cnt_psB, ones, cmp[:, 2:4, :].rearrange("p q e -> p (q e)"),
                    start=(b == 0), stop=(b == N_BIG - 1),
                )

            nc.sync.dma_start(out=tw_v, in_=tw_acc)
            nc.sync.dma_start(out=ids_v, in_=ids_acc)

            cnt_sb = singles.tile([1, Q * N_EXPERTS], F32)
            nc.vector.tensor_copy(cnt_sb[:, 0 : 2 * N_EXPERTS], cnt_psA)
            nc.vector.tensor_copy(cnt_sb[:, 2 * N_EXPERTS : 4 * N_EXPERTS], cnt_psB)
            nc.sync.dma_start(out=cnt_d.ap(), in_=cnt_sb)

    nc.compile()
    return nc


def _ensure_ntff_hook():
    """The agent image's antenv lacks axon_hooks; fabricate it and register
    the ctypes NTFF hook so trace=True produces a profile under axon."""
    import types

    import antenv

    if "antenv.axon_hooks" in sys.modules:
        return
    sys.path.insert(0, "/root/.axon_site")
    from trn_agent_boot.trn_boot import _ntff_profile_via_ctypes

    mod = types.ModuleType("antenv.axon_hooks")
    holder = [None]
    mod.set_axon_ntff_profile_hook = lambda h: holder.__setitem__(0, h)
    mod.get_axon_ntff_profile_hook = lambda: holder[0]
    sys.modules["antenv.axon_hooks"] = mod
    antenv.axon_hooks = mod
    mod.set_axon_ntff_profile_hook(
        _ntff_profile_via_ctypes("/opt/axon/libaxon_pjrt.so")
    )


_NC_CACHE = None


def _get_nc():
    global _NC_CACHE
    if _NC_CACHE is None:
        _NC_CACHE = _build_program()
    return _NC_CACHE


def _run(logits, trace=False, tmpdir=None):
    logits = np.ascontiguousarray(np.asarray(logits, dtype=np.float32))
    assert logits.shape == (SEQ, N_EXPERTS)
    nc = _get_nc()
    if trace:
        _ensure_ntff_hook()
    # core c gets tokens [c*16384, (c+1)*16384); inside, token t of the
    # shard lives at dram row t, program reads it partition-major.
    in_maps = [
        {"logits": logits[c * TOK_PER_CORE : (c + 1) * TOK_PER_CORE]}
        for c in range(N_CORES)
    ]
    res = run_bass_kernel_spmd(
        nc, in_maps, list(range(N_CORES)), trace=trace, tmpdir=tmpdir
    )
    outs = res.results
    rw = np.concatenate([r["rw"] for r in outs], axis=0)
    tw = np.concatenate([r["tw"] for r in outs], axis=0)
    ids = np.concatenate([r["ids"] for r in outs], axis=0).astype(np.int32)
    cnt = np.sum(
        [r["cnt"].reshape(Q, N_EXPERTS).sum(axis=0) for r in outs],
        axis=0,
        dtype=np.float64,
    ).reshape(N_EXPERTS)

    # max_index searches the whole 256-wide row, so an exact f32 duplicate of
    # a group max located in a *different* group can steal the match. Such an
    # id is out of its group -> detectable; fix id and count delta exactly.
    grp = np.arange(N_GROUPS, dtype=np.int32)[None, :]
    bad_r, bad_g = np.nonzero((ids >> 5) != grp)
    for r, g in zip(bad_r.tolist(), bad_g.tolist()):
        seg = logits[r, g * GROUP_SIZE : (g + 1) * GROUP_SIZE]
        new = g * GROUP_SIZE + int(np.argmax(seg))
        ids[r, g] = new
        # the out-of-group id never matched iota in group g's slice, so the
        # device counted nothing for this slot: just add the fixed one.
        cnt[new] += 1.0

    cnt = cnt.astype(np.float32)
    return (logits, rw, tw, ids, cnt), res


def kernel(**inputs):
    (out, _res) = _run(inputs["logits"], trace=False)
    return out


# revision 20
# speedup vs baseline: 3.3210x; 1.0251x over previous
"""Trainium2 Bass kernel for GreedyGroupedRouter (MoE routing).

Reference computation per token (row of 256 logits):
  rw   = softmax(logits)                          [S, 256]
  per group g of 32 experts: pick argmax           -> topk_ids [S, 8]
  tw   = gathered rw, renormalized to sum 1        [S, 8]
  cnt  = bincount(topk_ids, 256)                   [256]
Outputs: (logits, rw, tw, ids, cnt).  logits is a pass-through.

Sharding: token dim across 8 cores (16384 tokens/core), SPMD identical
program; tokens_per_expert partial counts summed on host.

Per-core program: 32 "big tiles" of 512 tokens laid out as SBUF
[128 partitions, 4 sub-rows, 256 experts] (token = p*128 + b*4 + q, i.e.
partition-major so every DMA run is >=4KB contiguous):
  DVE : group-max (8 groups of 32), max_index -> global expert ids,
        reciprocals
  ACT : exp (softmax numerator; no max-subtract needed, |logits|<~6),
        accum_out gives row sums for free, scaled copies
  Pool: is_equal(logits, groupmax broadcast) -> one-hot selection mask
  PE  : ones[128,1].T @ mask accumulated in PSUM -> per-expert counts
  small outputs (tw, ids) accumulate in SBUF, one big DMA at the end.
"""

import os
import sys

import numpy as np

sys.path.insert(0, "/opt/trn_rl_repo")

import concourse.bacc as bacc
import concourse.bass as bass
import concourse.tile as tile
from concourse import mybir
from concourse._compat import get_trn_type
from concourse.bass_utils import run_bass_kernel_spmd

SEQ = 131072
N_EXPERTS = 256
N_GROUPS = 8
GROUP_SIZE = 32
TOP_K = 8
N_CORES = 8
TOK_PER_CORE = SEQ // N_CORES  # 16384
Q = 4  # sub-rows per partition per big tile
P = 128
BIG_TOK = P * Q  # 512 tokens per big tile
N_BIG = TOK_PER_CORE // BIG_TOK  # 32
N_LOC = TOK_PER_CORE // P  # 128 tokens per partition

F32 = mybir.dt.float32
U32 = mybir.dt.uint32
U16 = mybir.dt.uint16
BF16 = mybir.dt.bfloat16
I32 = mybir.dt.int32
AX = mybir.AxisListType
ALU = mybir.AluOpType
ACTF = mybir.ActivationFunctionType


def _bcast_last(ap, n):
    """Append a broadcast (step 0) innermost dim of size n to an AP."""
    return bass.AP(tensor=ap.tensor, offset=ap.offset, ap=list(ap.ap) + [[0, n]])


def _build_program():
    nc = bacc.Bacc(
        get_trn_type() or "TRN2",
        target_bir_lowering=False,
        debug=False,
        num_devices=N_CORES,
    )

    logits_d = nc.dram_tensor("logits", [TOK_PER_CORE, N_EXPERTS], F32, kind="ExternalInput")
    rw_d = nc.dram_tensor("rw", [TOK_PER_CORE, N_EXPERTS], F32, kind="ExternalOutput")
    tw_d = nc.dram_tensor("tw", [TOK_PER_CORE, TOP_K], F32, kind="ExternalOutput")
    ids_d = nc.dram_tensor("ids", [TOK_PER_CORE, TOP_K], U16, kind="ExternalOutput")
    # 4 sub-row blocks of partial counts; host sums them
    cnt_d = nc.dram_tensor("cnt", [1, Q * N_EXPERTS], F32, kind="ExternalOutput")

    # token t = p*N_LOC + n  (partition-major)  ->  dram row view [p, n, e]
    lg_v = logits_d.ap().rearrange("(p n) e -> p n e", p=P)
    rw_v = rw_d.ap().rearrange("(p n) e -> p n e", p=P)
    tw_v = tw_d.ap().rearrange("(p n) g -> p n g", p=P)
    ids_v = ids_d.ap().rearrange("(p n) g -> p n g", p=P)

    with tile.TileContext(nc) as tc:
        with (
            tc.tile_pool(name="singles", bufs=1) as singles,
            tc.tile_pool(name="acc", bufs=1) as acc,
            tc.tile_pool(name="big", bufs=4) as bigp,
            tc.tile_pool(name="small", bufs=6) as smallp,
            tc.tile_pool(name="psum", bufs=1, space="PSUM") as psump,
        ):
            ones = singles.tile([P, 1], BF16)
            nc.vector.memset(ones, 1.0)
            # global expert index per row position, same on every partition
            iota_u = singles.tile([P, N_EXPERTS], U16)
            nc.gpsimd.iota(iota_u, pattern=[[1, N_EXPERTS]], base=0, channel_multiplier=0)
            iota = singles.tile([P, N_EXPERTS], BF16)
            nc.vector.tensor_copy(iota, iota_u)

            tw_acc = acc.tile([P, N_LOC, TOP_K], F32)
            ids_acc = acc.tile([P, N_LOC, TOP_K], U16)
            cnt_psA = psump.tile([1, 2 * N_EXPERTS], F32)
            cnt_psB = psump.tile([1, 2 * N_EXPERTS], F32)

            for b in range(N_BIG):
                L = bigp.tile([P, Q, N_EXPERTS], F32, tag="L")
                nc.sync.dma_start(out=L, in_=lg_v[:, b * Q : (b + 1) * Q, :])

                # group maxes for all Q sub-rows: [P, Q*8]
                GM = smallp.tile([P, Q * N_GROUPS], F32, tag="GM")
                nc.vector.reduce_max(
                    GM,
                    L[:, :, :].rearrange("p q (g s) -> p q g s", s=GROUP_SIZE),
                    axis=AX.X,
                )

                E = bigp.tile([P, Q, N_EXPERTS], F32, tag="E")
                rowsum = smallp.tile([P, Q], F32, tag="rowsum")
                for q in range(Q):
                    nc.scalar.activation(
                        E[:, q, :], L[:, q, :], ACTF.Exp,
                        accum_out=rowsum[:, q : q + 1],
                    )

                recip = smallp.tile([P, Q], F32, tag="recip")
                nc.vector.reciprocal(recip, rowsum)

                RW = bigp.tile([P, Q, N_EXPERTS], F32, tag="RW")
                for q in range(Q):
                    # balance the normalization multiply across ACT and DVE
                    if q < 3:
                        nc.scalar.activation(
                            RW[:, q, :], E[:, q, :], ACTF.Copy,
                            scale=recip[:, q : q + 1],
                        )
                    else:
                        nc.vector.tensor_scalar_mul(
                            RW[:, q, :], E[:, q, :], recip[:, q : q + 1]
                        )
                nc.sync.dma_start(out=rw_v[:, b * Q : (b + 1) * Q, :], in_=RW)

                # global expert ids: first index of each group max in the row
                for q in range(Q):
                    nc.vector.max_index(
                        ids_acc[:, b * Q + q, :],
                        GM[:, q * N_GROUPS : (q + 1) * N_GROUPS],
                        L[:, q, :],
                    )

                # normalized gathered weights: exp(gm) / sum_g exp(gm)
                EG = smallp.tile([P, Q * N_GROUPS], F32, tag="EG")
                nc.scalar.activation(EG, GM, ACTF.Exp)
                s8 = smallp.tile([P, Q], F32, tag="s8")
                nc.vector.reduce_sum(
                    s8, EG[:, :].rearrange("p (q g) -> p q g", g=N_GROUPS), axis=AX.X
                )
                r8 = smallp.tile([P, Q], F32, tag="r8")
                nc.vector.reciprocal(r8, s8)
                nc.vector.tensor_tensor(
                    tw_acc[:, b * Q : (b + 1) * Q, :],
                    EG[:, :].rearrange("p (q g) -> p q g", g=N_GROUPS),
                    _bcast_last(r8[:, :], N_GROUPS),
                    ALU.mult,
                )

                # exact one-hot of the chosen ids (== bincount of ids once
                # summed): cmp[p,q,g,s] = (32*g + s == ids[p,q,g])
                cmp = bigp.tile([P, Q, N_EXPERTS], BF16, tag="cmp")
                iota_ap = iota[:, :]
                iota_b = bass.AP(
                    tensor=iota_ap.tensor,
                    offset=iota_ap.offset,
                    ap=[iota_ap.ap[0], [0, Q], [GROUP_SIZE, N_GROUPS], [1, GROUP_SIZE]],
                )
                ids_bf = smallp.tile([P, Q * TOP_K], BF16, tag="ids_bf")
                nc.gpsimd.tensor_copy(
                    ids_bf,
                    ids_acc[:, b * Q : (b + 1) * Q, :].rearrange("p q g -> p (q g)"),
                )
                ids_b = _bcast_last(
                    ids_bf[:, :].rearrange("p (q g) -> p q g", g=N_GROUPS), GROUP_SIZE
                )
                nc.vector.tensor_tensor(
                    cmp[:, :, :].rearrange("p q (g s) -> p q g s", s=GROUP_SIZE),
                    iota_b,
                    ids_b,
                    ALU.is_equal,
                )
                nc.tensor.matmul(
                    cnt_psA, ones, cmp[:, 0:2, :].rearrange("p q e -> p (q e)"),
                    start=(b == 0), stop=(b == N_BIG - 1),
                )
                nc.tensor.matmul(
                    cnt_psB, ones, cmp[:, 2:4, :].rearrange("p q e -> p (q e)"),
                    start=(b == 0), stop=(b == N_BIG - 1),
                )

            nc.sync.dma_start(out=tw_v, in_=tw_acc)
            nc.sync.dma_start(out=ids_v, in_=ids_acc)

            cnt_sb = singles.tile([1, Q * N_EXPERTS], F32)
            nc.vector.tensor_copy(cnt_sb[:, 0 : 2 * N_EXPERTS], cnt_psA)
            nc.vector.tensor_copy(cnt_sb[:, 2 * N_EXPERTS : 4 * N_EXPERTS], cnt_psB)
            nc.sync.dma_start(out=cnt_d.ap(), in_=cnt_sb)

    nc.compile()
    return nc


def _ensure_ntff_hook():
    """The agent image's antenv lacks axon_hooks; fabricate it and register
    the ctypes NTFF hook so trace=True produces a profile under axon."""
    import types

    import antenv

    if "antenv.axon_hooks" in sys.modules:
        return
    sys.path.insert(0, "/root/.axon_site")
    from trn_agent_boot.trn_boot import _ntff_profile_via_ctypes

    mod = types.ModuleType("antenv.axon_hooks")
    holder = [None]
    mod.set_axon_ntff_profile_hook = lambda h: holder.__setitem__(0, h)
    mod.get_axon_ntff_profile_hook = lambda: holder[0]
    sys.modules["antenv.axon_hooks"] = mod
    antenv.axon_hooks = mod
    mod.set_axon_ntff_profile_hook(
        _ntff_profile_via_ctypes("/opt/axon/libaxon_pjrt.so")
    )


_NC_CACHE = None


def _get_nc():
    global _NC_CACHE
    if _NC_CACHE is None:
        _NC_CACHE = _build_program()
    return _NC_CACHE


def _run(logits, trace=False, tmpdir=None):
    logits = np.ascontiguousarray(np.asarray(logits, dtype=np.float32))
    assert logits.shape == (SEQ, N_EXPERTS)
    nc = _get_nc()
    if trace:
        _ensure_ntff_hook()
    # core c gets tokens [c*16384, (c+1)*16384); inside, token t of the
    # shard lives at dram row t, program reads it partition-major.
    in_maps = [
        {"logits": logits[c * TOK_PER_CORE : (c + 1) * TOK_PER_CORE]}
        for c in range(N_CORES)
    ]
    res = run_bass_kernel_spmd(
        nc, in_maps, list(range(N_CORES)), trace=trace, tmpdir=tmpdir
    )
    outs = res.results
    rw = np.concatenate([r["rw"] for r in outs], axis=0)
    tw = np.concatenate([r["tw"] for r in outs], axis=0)
    ids = np.concatenate([r["ids"] for r in outs], axis=0).astype(np.int32)
    cnt = np.sum(
        [r["cnt"].reshape(Q, N_EXPERTS).sum(axis=0) for r in outs],
        axis=0,
        dtype=np.float64,
    ).reshape(N_EXPERTS)

    # max_index searches the whole 256-wide row, so an exact f32 duplicate of
    # a group max located in a *different* group can steal the match. Such an
    # id is out of its group -> detectable; fix id and count delta exactly.
    grp = np.arange(N_GROUPS, dtype=np.int32)[None, :]
    bad_r, bad_g = np.nonzero((ids >> 5) != grp)
    for r, g in zip(bad_r.tolist(), bad_g.tolist()):
        seg = logits[r, g * GROUP_SIZE : (g + 1) * GROUP_SIZE]
        new = g * GROUP_SIZE + int(np.argmax(seg))
        ids[r, g] = new
        # the out-of-group id never matched iota in group g's slice, so the
        # device counted nothing for this slot: just add the fixed one.
        cnt[new] += 1.0

    cnt = cnt.astype(np.float32)
    return (logits, rw, tw, ids, cnt), res


def kernel(**inputs):
    (out, _res) = _run(inputs["logits"], trace=False)
    return out


# revision 23
# speedup vs baseline: 3.3323x; 1.0034x over previous
"""Trainium2 Bass kernel for GreedyGroupedRouter (MoE routing).

Reference computation per token (row of 256 logits):
  rw   = softmax(logits)                          [S, 256]
  per group g of 32 experts: pick argmax           -> topk_ids [S, 8]
  tw   = gathered rw, renormalized to sum 1        [S, 8]
  cnt  = bincount(topk_ids, 256)                   [256]
Outputs: (logits, rw, tw, ids, cnt).  logits is a pass-through.

Sharding: token dim across 8 cores (16384 tokens/core), SPMD identical
program; tokens_per_expert partial counts summed on host.

Per-core program: 32 "big tiles" of 512 tokens laid out as SBUF
[128 partitions, 4 sub-rows, 256 experts] (token = p*128 + b*4 + q, i.e.
partition-major so every DMA run is >=4KB contiguous):
  DVE : group-max (8 groups of 32), max_index -> global expert ids,
        reciprocals
  ACT : exp (softmax numerator; no max-subtract needed, |logits|<~6),
        accum_out gives row sums for free, scaled copies
  Pool: is_equal(logits, groupmax broadcast) -> one-hot selection mask
  PE  : ones[128,1].T @ mask accumulated in PSUM -> per-expert counts
  small outputs (tw, ids) accumulate in SBUF, one big DMA at the end.
"""

import os
import sys

import numpy as np

sys.path.insert(0, "/opt/trn_rl_repo")

import concourse.bacc as bacc
import concourse.bass as bass
import concourse.tile as tile
from concourse import mybir
from concourse._compat import get_trn_type
from concourse.bass_utils import run_bass_kernel_spmd

SEQ = 131072
N_EXPERTS = 256
N_GROUPS = 8
GROUP_SIZE = 32
TOP_K = 8
N_CORES = 8
TOK_PER_CORE = SEQ // N_CORES  # 16384
Q = 4  # sub-rows per partition per big tile
P = 128
BIG_TOK = P * Q  # 512 tokens per big tile
N_BIG = TOK_PER_CORE // BIG_TOK  # 32
N_LOC = TOK_PER_CORE // P  # 128 tokens per partition

F32 = mybir.dt.float32
U32 = mybir.dt.uint32
U16 = mybir.dt.uint16
BF16 = mybir.dt.bfloat16
I32 = mybir.dt.int32
AX = mybir.AxisListType
ALU = mybir.AluOpType
ACTF = mybir.ActivationFunctionType


def _bcast_last(ap, n):
    """Append a broadcast (step 0) innermost dim of size n to an AP."""
    return bass.AP(tensor=ap.tensor, offset=ap.offset, ap=list(ap.ap) + [[0, n]])


def _build_program():
    nc = bacc.Bacc(
        get_trn_type() or "TRN2",
        target_bir_lowering=False,
        debug=False,
        num_devices=N_CORES,
    )

    logits_d = nc.dram_tensor("logits", [TOK_PER_CORE, N_EXPERTS], F32, kind="ExternalInput")
    rw_d = nc.dram_tensor("rw", [TOK_PER_CORE, N_EXPERTS], F32, kind="ExternalOutput")
    tw_d = nc.dram_tensor("tw", [TOK_PER_CORE, TOP_K], F32, kind="ExternalOutput")
    ids_d = nc.dram_tensor("ids", [TOK_PER_CORE, TOP_K], U16, kind="ExternalOutput")
    # 4 sub-row blocks of partial counts; host sums them
    cnt_d = nc.dram_tensor("cnt", [1, Q * N_EXPERTS], F32, kind="ExternalOutput")

    # token t = p*N_LOC + n  (partition-major)  ->  dram row view [p, n, e]
    lg_v = logits_d.ap().rearrange("(p n) e -> p n e", p=P)
    rw_v = rw_d.ap().rearrange("(p n) e -> p n e", p=P)
    tw_v = tw_d.ap().rearrange("(p n) g -> p n g", p=P)
    ids_v = ids_d.ap().rearrange("(p n) g -> p n g", p=P)

    with tile.TileContext(nc) as tc:
        with (
            tc.tile_pool(name="singles", bufs=1) as singles,
            tc.tile_pool(name="acc", bufs=1) as acc,
            tc.tile_pool(name="big", bufs=4) as bigp,
            tc.tile_pool(name="small", bufs=6) as smallp,
            tc.tile_pool(name="psum", bufs=1, space="PSUM") as psump,
        ):
            ones = singles.tile([P, 1], BF16)
            nc.vector.memset(ones, 1.0)
            # global expert index per row position, same on every partition
            iota_u = singles.tile([P, N_EXPERTS], U16)
            nc.gpsimd.iota(iota_u, pattern=[[1, N_EXPERTS]], base=0, channel_multiplier=0)
            iota = singles.tile([P, N_EXPERTS], BF16)
            nc.vector.tensor_copy(iota, iota_u)

            tw_acc = acc.tile([P, N_LOC, TOP_K], F32)
            ids_acc = acc.tile([P, N_LOC, TOP_K], U16)
            cnt_psA = psump.tile([1, 2 * N_EXPERTS], F32)
            cnt_psB = psump.tile([1, 2 * N_EXPERTS], F32)

            pipeline = []
            for b in range(N_BIG):
                L = bigp.tile([P, Q, N_EXPERTS], F32, tag="L")
                nc.sync.dma_start(out=L, in_=lg_v[:, b * Q : (b + 1) * Q, :])

                # group maxes for all Q sub-rows: [P, Q*8]
                GM = smallp.tile([P, Q * N_GROUPS], F32, tag="GM")
                nc.vector.reduce_max(
                    GM,
                    L[:, :, :].rearrange("p q (g s) -> p q g s", s=GROUP_SIZE),
                    axis=AX.X,
                )

                E = bigp.tile([P, Q, N_EXPERTS], F32, tag="E")
                rowsum = smallp.tile([P, Q], F32, tag="rowsum")
                for q in range(Q):
                    nc.scalar.activation(
                        E[:, q, :], L[:, q, :], ACTF.Exp,
                        accum_out=rowsum[:, q : q + 1],
                    )

                recip = smallp.tile([P, Q], F32, tag="recip")
                nc.vector.reciprocal(recip, rowsum)

                # normalize the PREVIOUS tile's exp (recip for it is long
                # ready) so ACT never stalls on the exp->rowsum->recip
                # round-trip of the current tile.
                pipeline.append((b, E, recip))
                if len(pipeline) > 1:
                    pb, pE, precip = pipeline.pop(0)
                    RW = bigp.tile([P, Q, N_EXPERTS], F32, tag="RW")
                    for q in range(Q):
                        # balance the normalization multiply across ACT and DVE
                        if q < 3:
                            nc.scalar.activation(
                                RW[:, q, :], pE[:, q, :], ACTF.Copy,
                                scale=precip[:, q : q + 1],
                            )
                        else:
                            nc.vector.tensor_scalar_mul(
                                RW[:, q, :], pE[:, q, :], precip[:, q : q + 1]
                            )
                    nc.sync.dma_start(
                        out=rw_v[:, pb * Q : (pb + 1) * Q, :], in_=RW
                    )

                # global expert ids: first index of each group max in the row
                for q in range(Q):
                    nc.vector.max_index(
                        ids_acc[:, b * Q + q, :],
                        GM[:, q * N_GROUPS : (q + 1) * N_GROUPS],
                        L[:, q, :],
                    )

                # normalized gathered weights: exp(gm) / sum_g exp(gm)
                EG = smallp.tile([P, Q * N_GROUPS], F32, tag="EG")
                nc.scalar.activation(EG, GM, ACTF.Exp)
                s8 = smallp.tile([P, Q], F32, tag="s8")
                nc.vector.reduce_sum(
                    s8, EG[:, :].rearrange("p (q g) -> p q g", g=N_GROUPS), axis=AX.X
                )
                r8 = smallp.tile([P, Q], F32, tag="r8")
                nc.vector.reciprocal(r8, s8)
                nc.vector.tensor_tensor(
                    tw_acc[:, b * Q : (b + 1) * Q, :],
                    EG[:, :].rearrange("p (q g) -> p q g", g=N_GROUPS),
                    _bcast_last(r8[:, :], N_GROUPS),
                    ALU.mult,
                )

                # exact one-hot of the chosen ids (== bincount of ids once
                # summed): cmp[p,q,g,s] = (32*g + s == ids[p,q,g])
                cmp = bigp.tile([P, Q, N_EXPERTS], BF16, tag="cmp")
                iota_ap = iota[:, :]
                iota_b = bass.AP(
                    tensor=iota_ap.tensor,
                    offset=iota_ap.offset,
                    ap=[iota_ap.ap[0], [0, Q], [GROUP_SIZE, N_GROUPS], [1, GROUP_SIZE]],
                )
                ids_bf = smallp.tile([P, Q * TOP_K], BF16, tag="ids_bf")
                nc.gpsimd.tensor_copy(
                    ids_bf,
                    ids_acc[:, b * Q : (b + 1) * Q, :].rearrange("p q g -> p (q g)"),
                )
                ids_b = _bcast_last(
                    ids_bf[:, :].rearrange("p (q g) -> p q g", g=N_GROUPS), GROUP_SIZE
                )
                nc.vector.tensor_tensor(
                    cmp[:, :, :].rearrange("p q (g s) -> p q g s", s=GROUP_SIZE),
                    iota_b,
                    ids_b,
                    ALU.is_equal,
                )
                nc.tensor.matmul(
                    cnt_psA, ones, cmp[:, 0:2, :].rearrange("p q e -> p (q e)"),
                    start=(b == 0), stop=(b == N_BIG - 1),
                )
                nc.tensor.matmul(
                    cnt_psB, ones, cmp[:, 2:4, :].rearrange("p q e -> p (q e)"),
                    start=(b == 0), stop=(b == N_BIG - 1),
                )

            # drain the software pipeline: normalize + store the last tile
            while pipeline:
                pb, pE, precip = pipeline.pop(0)
                RW = bigp.tile([P, Q, N_EXPERTS], F32, tag="RW")
                for q in range(Q):
                    if q < 3:
                        nc.scalar.activation(
                            RW[:, q, :], pE[:, q, :], ACTF.Copy,
                            scale=precip[:, q : q + 1],
                        )
                    else:
                        nc.vector.tensor_scalar_mul(
                            RW[:, q, :], pE[:, q, :], precip[:, q : q + 1]
                        )
                nc.sync.dma_start(out=rw_v[:, pb * Q : (pb + 1) * Q, :], in_=RW)

            nc.sync.dma_start(out=tw_v, in_=tw_acc)
            nc.sync.dma_start(out=ids_v, in_=ids_acc)

            cnt_sb = singles.tile([1, Q * N_EXPERTS], F32)
            nc.vector.tensor_copy(cnt_sb[:, 0 : 2 * N_EXPERTS], cnt_psA)
            nc.vector.tensor_copy(cnt_sb[:, 2 * N_EXPERTS : 4 * N_EXPERTS], cnt_psB)
            nc.sync.dma_start(out=cnt_d.ap(), in_=cnt_sb)

    nc.compile()
    return nc


def _ensure_ntff_hook():
    """The agent image's antenv lacks axon_hooks; fabricate it and register
    the ctypes NTFF hook so trace=True produces a profile under axon."""
    import types

    import antenv

    if "antenv.axon_hooks" in sys.modules:
        return
    sys.path.insert(0, "/root/.axon_site")
    from trn_agent_boot.trn_boot import _ntff_profile_via_ctypes

    mod = types.ModuleType("antenv.axon_hooks")
    holder = [None]
    mod.set_axon_ntff_profile_hook = lambda h: holder.__setitem__(0, h)
    mod.get_axon_ntff_profile_hook = lambda: holder[0]
    sys.modules["antenv.axon_hooks"] = mod
    antenv.axon_hooks = mod
    mod.set_axon_ntff_profile_hook(
        _ntff_profile_via_ctypes("/opt/axon/libaxon_pjrt.so")
    )


_NC_CACHE = None


def _get_nc():
    global _NC_CACHE
    if _NC_CACHE is None:
        _NC_CACHE = _build_program()
    return _NC_CACHE


def _run(logits, trace=False, tmpdir=None):
    logits = np.ascontiguousarray(np.asarray(logits, dtype=np.float32))
    assert logits.shape == (SEQ, N_EXPERTS)
    nc = _get_nc()
    if trace:
        _ensure_ntff_hook()
    # core c gets tokens [c*16384, (c+1)*16384); inside, token t of the
    # shard lives at dram row t, program reads it partition-major.
    in_maps = [
        {"logits": logits[c * TOK_PER_CORE : (c + 1) * TOK_PER_CORE]}
        for c in range(N_CORES)
    ]
    res = run_bass_kernel_spmd(
        nc, in_maps, list(range(N_CORES)), trace=trace, tmpdir=tmpdir
    )
    outs = res.results
    rw = np.concatenate([r["rw"] for r in outs], axis=0)
    tw = np.concatenate([r["tw"] for r in outs], axis=0)
    ids = np.concatenate([r["ids"] for r in outs], axis=0).astype(np.int32)
    cnt = np.sum(
        [r["cnt"].reshape(Q, N_EXPERTS).sum(axis=0) for r in outs],
        axis=0,
        dtype=np.float64,
    ).reshape(N_EXPERTS)

    # max_index searches the whole 256-wide row, so an exact f32 duplicate of
    # a group max located in a *different* group can steal the match. Such an
    # id is out of its group -> detectable; fix id and count delta exactly.
    grp = np.arange(N_GROUPS, dtype=np.int32)[None, :]
    bad_r, bad_g = np.nonzero((ids >> 5) != grp)
    for r, g in zip(bad_r.tolist(), bad_g.tolist()):
        seg = logits[r, g * GROUP_SIZE : (g + 1) * GROUP_SIZE]
        new = g * GROUP_SIZE + int(np.argmax(seg))
        ids[r, g] = new
        # the out-of-group id never matched iota in group g's slice, so the
        # device counted nothing for this slot: just add the fixed one.
        cnt[new] += 1.0

    cnt = cnt.astype(np.float32)
    return (logits, rw, tw, ids, cnt), res


def kernel(**inputs):
    (out, _res) = _run(inputs["logits"], trace=False)
    return out
